# revision 26
# baseline (speedup 1.0000x reference)
"""ExpertNet (moe_routing) Trainium2 Bass kernel, v2.

Data-parallel over 8 NeuronCores: batch N=32768 split into 8 shards of 4096.
All parameters replicated.  Per-core pipeline per 512-sample block:

  encoder   48 fp8e4m3 DoubleRow matmuls (X and Wenc split hi+lo, 3-term
            error compensation: Xh*Wh + Xl*Wh + Xh*Wl, all scaled x2048,
            descaled in the relu's ACT scale) -> hts bf16
  z layer   4 bf16 matmuls -> zt2 (z duplicated on 128 partitions, bf16)
  q         dist via 3 matmuls (bd1*ones + (-2mu^T)z + ones^T zsq),
            qr = 1/dist on DVE; qsum broadcast to 32 partitions directly
            via a [16,32] ones lhsT; prb = 1/qsum on DVE
  qb        per-pair q broadcasts on the GpSimd/Pool engine (off the PE),
            zq = zt2*qb on DVE (bf16 2x mode)
  experts   32 hidden matmuls (row-packed pairs, zq bf16 rhs) -> relu
            (ACT/DVE split) -> 32 combine matmuls accumulating q-weighted
            logits + b2 term into one [32,512] PSUM
  tail      preds^T scaled by prb (DVE, reads PSUM), 32x32 transpose, DMA

All SBUF activations/weights are bf16 (fp8 for the encoder operands);
PSUM stays fp32.  Host-side _prep quantizes X/Wenc to fp8 hi+lo with
power-of-2 scales (Xh*32, Xl*32, Wh*64, Wl*64 -> products x2048).

Fronts run `ahead` blocks before their backs so encoder work fills the
q-chain and relu dependency stalls.  b1 != 0 falls back to the v1 path.
"""

import numpy as np

N, D, H_ENC, NZ, KE, H_EXP, C = 32768, 1024, 512, 64, 16, 256, 10
NCORES = 8
NS = N // NCORES          # samples per core
NB = 512                  # samples per block (matmul moving free dim)
NBLK = NS // NB
NPAIR = KE // 2           # expert pairs (row-packed)

_CACHE = {}
LAST_RESULTS = None


def _build(has_b1: bool, cfg: dict | None = None, has_b2: bool = False):
    if has_b1:
        return _build_v1(has_b1, cfg)
    defaults = dict(pbig=4, pmisc=1, ppred=2, pqb=1, hbufs=9, ehbufs=12,
                    zqbufs=32, xbufs=6, repeat=1, W=NB, ahead=2, xahead=3,
                    act_mod=8, act_lt=5, lag=5, qj=3)
    cfg = {**defaults, **(cfg or {})}
    import concourse.bacc as bacc
    import concourse.mybir as mybir
    from concourse import tile

    F32 = mybir.dt.float32
    F32R = mybir.dt.float32r
    BF16 = mybir.dt.bfloat16
    F8 = mybir.dt.float8e4
    AF = mybir.ActivationFunctionType
    DR = mybir.MatmulPerfMode.DoubleRow

    W = cfg["W"]
    nc = bacc.Bacc("TRN2", target_bir_lowering=False, debug=False,
                   num_devices=NCORES)

    # ---- I/O ----------------------------------------------------------
    XH8 = nc.dram_tensor("XH8", [128, 8, NS], F8, kind="ExternalInput")
    XL8 = nc.dram_tensor("XL8", [128, 8, NS], F8, kind="ExternalInput")
    WENH = nc.dram_tensor("WENH", [128, 8, 4, 128], F8, kind="ExternalInput")
    WENL = nc.dram_tensor("WENL", [128, 8, 4, 128], F8, kind="ExternalInput")
    WZ = nc.dram_tensor("WZ", [128, 4, NZ], BF16, kind="ExternalInput")
    DCOMBO = nc.dram_tensor("DCOMBO", [128, 2, KE], F8, kind="ExternalInput")
    O1632 = nc.dram_tensor("O1632", [KE, 32], BF16, kind="ExternalInput")
    B2PAD = nc.dram_tensor("B2PAD", [KE, 32], BF16, kind="ExternalInput")
    E2 = nc.dram_tensor("E2", [KE, NPAIR * 128], BF16, kind="ExternalInput")
    W1P = nc.dram_tensor("W1P", [128, NPAIR * H_EXP], BF16,
                         kind="ExternalInput")
    W2C = nc.dram_tensor("W2C", [128, KE * 2 * 32], BF16,
                         kind="ExternalInput")
    BENC = nc.dram_tensor("BENC", [128, 4], F32, kind="ExternalInput")
    BZ = nc.dram_tensor("BZ", [NZ, 1], F32, kind="ExternalInput")
    OUT = nc.dram_tensor("OUT", [NS, C], F32, kind="ExternalOutput")

    with tile.TileContext(nc) as tc, nc.allow_low_precision(
        reason="bf16/fp8 operands, validated 5.9e-3 max rel err"
    ):
        with (
            tc.tile_pool(name="wpool", bufs=1) as wp,
            tc.tile_pool(name="xpool", bufs=cfg["xbufs"]) as xp,
            tc.tile_pool(name="hpool", bufs=cfg["hbufs"]) as hp,
            tc.tile_pool(name="zpool", bufs=5) as zp,
            tc.tile_pool(name="qpool", bufs=6) as qp,
            tc.tile_pool(name="zqpool", bufs=cfg["zqbufs"]) as zqp,
            tc.tile_pool(name="ehpool", bufs=cfg["ehbufs"]) as ehp,
            tc.tile_pool(name="trpool", bufs=2) as trp,
            tc.tile_pool(name="pbig", bufs=cfg["pbig"], space="PSUM") as pbig,
            tc.tile_pool(name="pmisc", bufs=cfg["pmisc"], space="PSUM") as pmisc,
            tc.tile_pool(name="pqb", bufs=cfg["pqb"], space="PSUM") as pqb,
            tc.tile_pool(name="ppred", bufs=cfg["ppred"], space="PSUM") as ppred,
        ):
            def wload(dram, shape, dt):
                t = wp.tile(shape, dt, name=dram.name + "_sb")
                nc.sync.dma_start(t[:], dram[:])
                return t

            # front-critical DMA order: xh(0), wenh, xl(0), wenl, benc —
            # DMA *issue* on the SP sequencer is ~650ns each and serialized,
            # so block 0 uses one whole-tile DMA per tensor.
            xdma = {}

            def dma_x(ib):
                n0 = ib * NB
                xh = xp.tile([128, 8, NB], F8, tag="xh")
                xl = xp.tile([128, 8, NB], F8, tag="xl")
                nc.sync.dma_start(xh[:, :, :W], XH8[:, :, n0:n0 + W])
                nc.sync.dma_start(xl[:, :, :W], XL8[:, :, n0:n0 + W])
                xdma[ib] = (xh, xl)

            wenh = wp.tile([128, 8, 4, 128], F8, name="WENH_sb")
            nc.sync.dma_start(wenh[:, 0:4], WENH[:, 0:4])
            xh0 = xp.tile([128, 8, NB], F8, tag="xh")
            nc.sync.dma_start(xh0[:, 0:4, :W], XH8[:, 0:4, 0:W])
            nc.sync.dma_start(wenh[:, 4:8], WENH[:, 4:8])
            nc.sync.dma_start(xh0[:, 4:8, :W], XH8[:, 4:8, 0:W])
            xl0 = xp.tile([128, 8, NB], F8, tag="xl")
            nc.sync.dma_start(xl0[:, :, :W], XL8[:, :, 0:W])
            wenl = wp.tile([128, 8, 4, 128], F8, name="WENL_sb")
            nc.sync.dma_start(wenl[:], WENL[:])
            xdma[0] = (xh0, xl0)
            benc = wload(BENC, [128, 4], F32)
            wz = wload(WZ, [128, 4, NZ], BF16)
            bz = wload(BZ, [NZ, 1], F32)
            dcombo = wload(DCOMBO, [128, 2, KE], F8)
            o1632 = wload(O1632, [KE, 32], BF16)
            e2 = wload(E2, [KE, NPAIR * 128], BF16)

            late = {}

            def load_late_weights():
                late["w1p"] = wload(W1P, [128, NPAIR * H_EXP], BF16)
                late["w2c"] = wload(W2C, [128, KE * 2 * 32], BF16)
                if has_b2:
                    late["b2p"] = wload(B2PAD, [KE, 32], BF16)

            def front(ib):
                n0 = ib * NB
                xh, xl = xdma.pop(ib)

                # encoder: 12 DoubleRow matmuls per 128-out chunk
                hts = []
                for hc in range(4):
                    ph = pbig.tile([128, NB], F32, tag="pbig")
                    nmm = 0
                    for xt, wt in ((xh, wenh), (xl, wenh), (xh, wenl)):
                        for p in range(4):
                            nmm += 1
                            nc.tensor.matmul(
                                ph[:, :W],
                                wt[:, 2 * p:2 * p + 2, hc, :],
                                xt[:, 2 * p:2 * p + 2, :W],
                                start=(nmm == 1), stop=(nmm == 12),
                                perf_mode=DR,
                            )
                    ht = hp.tile([128, NB], BF16, tag="ht")
                    nc.scalar.activation(ht[:, :W], ph[:, :W], AF.Relu,
                                         bias=benc[:, hc:hc + 1],
                                         scale=1.0 / 2048.0)
                    hts.append(ht)

                # z layer: zT = Wz^T hT + bz
                pz = pmisc.tile([NZ, NB], F32, tag="pmisc")
                for hc in range(4):
                    nc.tensor.matmul(
                        pz[:, :W], wz[:, hc, :], hts[hc][:, :W],
                        start=(hc == 0), stop=(hc == 3),
                    )
                zt2 = zp.tile([128, NB], BF16, tag="zt2")
                nc.vector.tensor_copy(zt2[0:NZ, :W], pz[:, :W])
                # zcombo: ktile0 = z*32, ktile1 = (8z)^2 = 64 z^2, row 64 =
                # 128.0 carrier for the bd1 hi/lo terms.  One DoubleRow
                # matmul then computes 512*dist; the x512 cancels in the
                # q normalization.
                zc = zp.tile([128, 2, NB], F8, tag="zc")
                nc.vector.tensor_scalar_mul(zc[0:NZ, 0, :W], pz[:, :W], 32.0)
                nc.vector.tensor_mul(zc[0:NZ, 1, :W], zt2[0:NZ, :W],
                                     zt2[0:NZ, :W])
                nc.gpsimd.memset(zc[64:128, :, :W], 0.0)
                nc.gpsimd.memset(zc[64:66, :, :W], 128.0)
                nc.scalar.activation(zt2[NZ:128, :W], pz[:, :W], AF.Identity,
                                     bias=bz[:])
                return dict(zt2=zt2, zc=zc, n0=n0, zqs=[])

            def emit_q(st):
                # 512*dist in ONE DoubleRow matmul (z, z^2 and the bd1
                # carrier row packed as the two fp8 k-tiles).
                zc = st["zc"]
                pd = pmisc.tile([KE, NB], F32, tag="pmisc")
                nc.tensor.matmul(pd[:, :W], dcombo[:, :, :], zc[:, :, :W],
                                 start=True, stop=True, perf_mode=DR)
                qr = qp.tile([KE, NB], BF16, tag="qr")
                nc.vector.reciprocal(qr[:, :W], pd[:, :W])
                pqs = pmisc.tile([32, NB], F32, tag="pmisc")
                nc.tensor.matmul(pqs[:, :W], o1632[:], qr[:, :W],
                                 start=True, stop=True)
                prb = qp.tile([32, NB], F32, tag="prb")
                nc.vector.reciprocal(prb[:, :W], pqs[:, :W])
                st["qr"], st["prb"] = qr, prb

            def emit_zq(st, j):
                # qb broadcast: PE selector matmul, zq on DVE
                pqbt = pqb.tile([128, NB], F32, tag="pqb")
                nc.tensor.matmul(pqbt[:, :W], e2[:, j * 128:(j + 1) * 128],
                                 st["qr"][:, :W], start=True, stop=True)
                zq = zqp.tile([128, NB], BF16, tag="zq")
                nc.vector.tensor_mul(zq[:, :W], st["zt2"][:, :W], pqbt[:, :W])
                st["zqs"].append(zq)

            def back(st, fut=None, last=False):
                zqs, qr, prb, n0 = st["zqs"], st["qr"], st["prb"], st["n0"]
                nhalf = 1
                pps, nseen, hdone = [], [0] * nhalf, []
                for hb in range(nhalf):
                    pp = ppred.tile([32, NB], F32, tag="ppred",
                                    name=f"pp{hb}")
                    if has_b2 and hb == 0:
                        nc.tensor.matmul(pp[:, :W], late["b2p"][:], qr[:, :W],
                                         start=True, stop=False)
                    pps.append(pp)

                ncomb = NPAIR * 4
                nper = ncomb // nhalf
                ci = 0
                pend = []

                def combine(eh, idx):
                    nonlocal ci
                    ci += 1
                    hb = (idx % 2) if nhalf == 2 else 0
                    nseen[hb] += 1
                    first = nseen[hb] == 1 and not (has_b2 and hb == 0)
                    stop = nseen[hb] == nper
                    # w2c cols 10-31 are zero, so the first accumulate with
                    # start=True also zero-fills pp rows 10-31 exactly.
                    nc.tensor.matmul(
                        pps[hb][:, :W],
                        late["w2c"][:, idx * 32:(idx + 1) * 32],
                        eh[:, :W],
                        start=first, stop=stop,
                        skip_group_check=True,
                    )
                    if stop:
                        hdone.append(hb)

                for j in range(NPAIR):
                    zq = zqs[j]
                    if fut is not None:
                        if j == cfg["qj"]:
                            emit_q(fut)
                        elif j > cfg["qj"]:
                            emit_zq(fut, j - cfg["qj"] - 1)
                    for hc in range(2):
                        for half in range(2):
                            k = 2 * j + half
                            idx = k * 2 + hc
                            pe_ = pbig.tile([128, NB], F32, tag="pbig")
                            nc.tensor.matmul(
                                pe_[:, :W],
                                late["w1p"][64 * half:64 * (half + 1),
                                    j * H_EXP + hc * 128:
                                    j * H_EXP + (hc + 1) * 128],
                                zq[64 * half:64 * (half + 1), :W],
                                start=True, stop=True,
                                tile_position=(64 * half, 0),
                            )
                            eh = ehp.tile([128, NB], BF16, tag="eh")
                            if idx >= 28 or idx % 7 < 4:
                                nc.scalar.activation(eh[:, :W], pe_[:, :W],
                                                     AF.Relu, bias=0.0)
                            else:
                                nc.vector.tensor_scalar_max(eh[:, :W],
                                                            pe_[:, :W], 0.0)
                            pend.append((eh, idx))
                            lag = cfg["lag"] if fut is not None else cfg["lag"] + 2
                            if len(pend) > lag:
                                combine(*pend.pop(0))
                if fut is not None:
                    for jj in range(NPAIR - cfg["qj"] - 1, NPAIR):
                        emit_zq(fut, jj)
                for ehi in pend:
                    combine(*ehi)

                # normalize (reads PSUM directly), transpose, store; two
                # sample-halves so the tail chain is shorter
                H = W // 2
                for hb in range(2):
                    s = hb * H
                    ti = trp.tile([32, H], F32, tag="ti")
                    if nhalf == 2:
                        tsum = trp.tile([32, H], F32, tag="tsum")
                        nc.vector.tensor_add(tsum[:, :H], pps[0][:, s:s + H],
                                             pps[1][:, s:s + H])
                        nc.vector.tensor_mul(ti[:, :H], tsum[:, :H],
                                             prb[:, s:s + H])
                    else:
                        nc.vector.tensor_mul(ti[:, :H], pps[0][:, s:s + H],
                                             prb[:, s:s + H])
                    tr = trp.tile([32, H], F32, tag="tr")
                    nc.vector.transpose(tr[:, :H], ti[:, :H])
                    nc.sync.dma_start(
                        OUT[n0 + s:n0 + s + H, :].rearrange(
                            "(b p) c -> p b c", p=32),
                        tr[:].rearrange("p (b v) -> p b v", v=32)[:, 0:H // 32,
                                                                 0:C],
                    )

            A = cfg["ahead"]
            XA = cfg["xahead"]
            for _rep in range(cfg["repeat"]):
                for ib in range(1, min(XA, NBLK)):
                    if ib not in xdma:
                        dma_x(ib)
                sts = [front(0)]
                if _rep == 0 and not late:
                    load_late_weights()
                emit_q(sts[0])
                for j in range(NPAIR):
                    emit_zq(sts[0], j)
                for ib in range(1, min(A, NBLK)):
                    sts.append(front(ib))
                    emit_q(sts[ib])
                    for j in range(NPAIR):
                        emit_zq(sts[ib], j)
                for ib in range(NBLK):
                    if ib + XA < NBLK and (ib + XA) not in xdma:
                        dma_x(ib + XA)
                    fut = None
                    if ib + A < NBLK:
                        sts.append(front(ib + A))
                        fut = sts[ib + A]
                    back(sts[ib], fut=fut, last=(ib == NBLK - 1))
                sts.clear()
                if cfg["repeat"] > 1:
                    dma_x(0)

    nc.compile()
    return nc


def _prep(inputs):
    import ml_dtypes
    f8 = ml_dtypes.float8_e4m3
    bf = ml_dtypes.bfloat16
    f = lambda a: np.ascontiguousarray(np.asarray(a, dtype=np.float32))
    X, enc_W, enc_b = f(inputs["X"]), f(inputs["enc_W"]), f(inputs["enc_b"])
    z_W, z_b, mu = f(inputs["z_W"]), f(inputs["z_b"]), f(inputs["mu"])
    W1, b1, W2, b2 = f(inputs["W1"]), f(inputs["b1"]), f(inputs["W2"]), f(inputs["b2"])

    has_b1 = bool(np.any(b1))
    if has_b1 or np.any(z_b):
        # v2 drops the z bias in its DVE copies; route through v1.
        in_maps_v1, has_b1_real = _prep_v1(inputs)
        return in_maps_v1, ("v1", has_b1_real)

    # fp8 hi/lo split: Xh*32, Xl*32, Wh*64, Wl*64 -> every product x2048
    XT = X.T                                            # [D, N]
    XH = (XT * 32.0).astype(f8)
    XLf = (XT - XH.astype(np.float32) / 32.0) * 32.0
    XL = XLf.astype(f8)
    WH = (enc_W * 64.0).astype(f8)
    WLf = (enc_W - WH.astype(np.float32) / 64.0) * 64.0
    WL = WLf.astype(f8)

    def enc_w_pack(w8):                                 # [D, H_ENC] -> [128,8,4,128]
        return np.ascontiguousarray(
            w8.reshape(8, 128, 4, 128).transpose(1, 0, 2, 3))

    com = {
        "WENH": enc_w_pack(WH),
        "WENL": enc_w_pack(WL),
        "WZ": np.ascontiguousarray(
            z_W.reshape(4, 128, NZ).transpose(1, 0, 2)).astype(bf),
        "O1632": np.ones((KE, 32), bf),
        "BENC": np.ascontiguousarray(enc_b.reshape(4, 128).T),
        "BZ": z_b.reshape(NZ, 1).copy(),
    }
    bd1f = (1.0 + (mu.astype(np.float64) ** 2).sum(axis=1)).astype(np.float32)
    dcombo = np.zeros((128, 2, KE), np.float32)
    dcombo[0:NZ, 0, :] = -2.0 * mu.T * 4.0           # * (z*32) -> 128*(-2 z.mu)
    dcombo[0:NZ, 1, :] = 128.0                       # * z^2 -> 128*|z|^2
    # rows 64-65 hit the 128.0 carrier rows of zc: progressive-refinement
    # split of bd1 over four fp8 slots -> 128*bd1 (e4m3 max is 240)
    t = bd1f.astype(np.float64)
    w0 = t.astype(f8).astype(np.float64)
    w1 = (t - w0).astype(f8).astype(np.float64)
    w2 = (t - w0 - w1).astype(f8).astype(np.float64)
    dcombo[64, 0, :] = w0
    dcombo[64, 1, :] = w1
    dcombo[65, 0, :] = w2
    dcombo[65, 1, :] = (t - w0 - w1 - w2).astype(np.float32)
    com["DCOMBO"] = dcombo.astype(f8)
    w1p = np.zeros((128, NPAIR * H_EXP), np.float32)
    e2 = np.zeros((KE, NPAIR * 128), np.float32)
    for j in range(NPAIR):
        w1p[0:64, j * H_EXP:(j + 1) * H_EXP] = W1[2 * j]
        w1p[64:128, j * H_EXP:(j + 1) * H_EXP] = W1[2 * j + 1]
        e2[2 * j, j * 128: j * 128 + 64] = 1.0
        e2[2 * j + 1, j * 128 + 64: j * 128 + 128] = 1.0
    com["W1P"] = w1p.astype(bf)
    com["E2"] = e2.astype(bf)

    w2c = np.zeros((128, KE * 2 * 32), np.float32)
    for k in range(KE):
        for hc in range(2):
            w2c[:, (k * 2 + hc) * 32:(k * 2 + hc) * 32 + C] = \
                W2[k][hc * 128:(hc + 1) * 128, :]
    com["W2C"] = w2c.astype(bf)

    b2pad = np.zeros((KE, 32), np.float32)
    b2pad[:, 0:C] = b2
    com["B2PAD"] = b2pad.astype(bf)

    in_maps = []
    for c in range(NCORES):
        m = dict(com)
        m["XH8"] = np.ascontiguousarray(
            XH[:, c * NS:(c + 1) * NS].reshape(8, 128, NS).transpose(1, 0, 2))
        m["XL8"] = np.ascontiguousarray(
            XL[:, c * NS:(c + 1) * NS].reshape(8, 128, NS).transpose(1, 0, 2))
        in_maps.append(m)
    return in_maps, ("v2",)


def kernel(**inputs) -> np.ndarray:
    global LAST_RESULTS
    from concourse.bass_utils import run_bass_kernel_spmd

    in_maps, mode = _prep(inputs)
    has_b2 = bool(np.any(np.asarray(inputs["b2"])))
    if mode[0] == "v1":
        key = ("v1", mode[1])
        if key not in _CACHE:
            _CACHE[key] = _build_v1(mode[1])
    else:
        key = ("v2", has_b2)
        if key not in _CACHE:
            _CACHE[key] = _build(False, has_b2=has_b2)
    nc = _CACHE[key]

    res = run_bass_kernel_spmd(nc, in_maps, list(range(NCORES)))
    LAST_RESULTS = res
    out = np.concatenate([res.results[c]["OUT"] for c in range(NCORES)], axis=0)
    return np.ascontiguousarray(out, dtype=np.float32)


# ---------------------------------------------------------------------------
# v1 fallback (b1 != 0): original fp32r kernel
# ---------------------------------------------------------------------------

def _build_v1(has_b1: bool, cfg: dict | None = None):
    defaults = dict(pbig=4, pmisc=1, pqb=2, ppred=1, hbufs=9, ehbufs=10,
                    zqbufs=3, xbufs=3, qb_gpsimd=False, repeat=1, W=NB,
                    ahead=2)
    cfg = {**defaults, **(cfg or {})}
    import concourse.bacc as bacc
    import concourse.mybir as mybir
    from concourse import tile

    F32 = mybir.dt.float32
    F32R = mybir.dt.float32r
    AF = mybir.ActivationFunctionType

    W = cfg["W"]
    nc = bacc.Bacc("TRN2", target_bir_lowering=False, debug=False,
                   num_devices=NCORES)

    XT = nc.dram_tensor("XT", [8, 128, NS], F32R, kind="ExternalInput")
    Wenc = nc.dram_tensor("Wenc", [128, 8 * H_ENC], F32R, kind="ExternalInput")
    Wz = nc.dram_tensor("Wz", [128, 4 * NZ], F32R, kind="ExternalInput")
    W1p = nc.dram_tensor("W1p", [128, NPAIR * H_EXP], F32R, kind="ExternalInput")
    W2c = nc.dram_tensor("W2c", [128, KE * 2 * 32], F32R, kind="ExternalInput")
    NEG2MUT = nc.dram_tensor("NEG2MUT", [NZ, KE], F32R, kind="ExternalInput")
    ONES64 = nc.dram_tensor("ONES64", [NZ, KE], F32R, kind="ExternalInput")
    ONES16 = nc.dram_tensor("ONES16", [KE, 1], F32R, kind="ExternalInput")
    ONES1_32 = nc.dram_tensor("ONES1_32", [1, 32], F32R, kind="ExternalInput")
    ONESN = nc.dram_tensor("ONESN", [1, NB], F32R, kind="ExternalInput")
    E2 = nc.dram_tensor("E2", [KE, NPAIR * 128], F32R, kind="ExternalInput")
    B2PAD = nc.dram_tensor("B2PAD", [KE, 32], F32R, kind="ExternalInput")
    BENC = nc.dram_tensor("BENC", [128, 4], F32, kind="ExternalInput")
    BZ = nc.dram_tensor("BZ", [NZ, 1], F32, kind="ExternalInput")
    BD1 = nc.dram_tensor("BD1", [1, KE], F32R, kind="ExternalInput")
    if has_b1:
        B1C = nc.dram_tensor("B1C", [128, KE * 2], F32, kind="ExternalInput")
        E2S = nc.dram_tensor("E2S", [KE, KE * 128], F32R, kind="ExternalInput")
    OUT = nc.dram_tensor("OUT", [NS, C], F32, kind="ExternalOutput")

    with tile.TileContext(nc) as tc, nc.allow_low_precision(
        reason="float32r tiles feed the PE; rounding is ~1e-4 relative"
    ):
        with (
            tc.tile_pool(name="wpool", bufs=1) as wp,
            tc.tile_pool(name="xpool", bufs=cfg["xbufs"]) as xp,
            tc.tile_pool(name="hpool", bufs=cfg["hbufs"]) as hp,
            tc.tile_pool(name="zpool", bufs=2) as zp,
            tc.tile_pool(name="qpool", bufs=2) as qp,
            tc.tile_pool(name="zqpool", bufs=cfg["zqbufs"]) as zqp,
            tc.tile_pool(name="ehpool", bufs=cfg["ehbufs"]) as ehp,
            tc.tile_pool(name="trpool", bufs=2) as trp,
            tc.tile_pool(name="pbig", bufs=cfg["pbig"], space="PSUM") as pbig,
            tc.tile_pool(name="pmisc", bufs=cfg["pmisc"], space="PSUM") as pmisc,
            tc.tile_pool(name="pqb", bufs=max(cfg["pqb"], 1), space="PSUM") as pqb,
            tc.tile_pool(name="ppred", bufs=cfg["ppred"], space="PSUM") as ppred,
        ):
            def wload(dram, shape, dt):
                t = wp.tile(shape, dt, name=dram.name + "_sb")
                nc.sync.dma_start(t[:], dram[:])
                return t

            wenc = wp.tile([128, 8 * H_ENC], F32R, name="Wenc_sb")
            for dc in range(8):
                nc.sync.dma_start(wenc[:, dc * H_ENC:(dc + 1) * H_ENC],
                                  Wenc[:, dc * H_ENC:(dc + 1) * H_ENC])
            benc = wload(BENC, [128, 4], F32)
            wz = wload(Wz, [128, 4 * NZ], F32R)
            n2mu = wload(NEG2MUT, [NZ, KE], F32R)
            o64 = wload(ONES64, [NZ, KE], F32R)
            o16 = wload(ONES16, [KE, 1], F32R)
            o132 = wload(ONES1_32, [1, 32], F32R)
            onesn = wload(ONESN, [1, NB], F32R)
            bz = wload(BZ, [NZ, 1], F32)
            bd1 = wload(BD1, [1, KE], F32R)

            late = {}

            def load_late_weights():
                late["w1p"] = wload(W1p, [128, NPAIR * H_EXP], F32R)
                late["w2c"] = wload(W2c, [128, KE * 2 * 32], F32R)
                late["e2"] = wload(E2, [KE, NPAIR * 128], F32R)
                late["b2p"] = wload(B2PAD, [KE, 32], F32R)
                if has_b1:
                    late["b1c"] = wload(B1C, [128, KE * 2], F32)
                    late["e2s"] = wload(E2S, [KE, KE * 128], F32R)

            def front(ib):
                n0 = ib * NB
                xt = xp.tile([128, 8 * NB], F32R, tag="xt")
                for dc in range(8):
                    nc.sync.dma_start(
                        xt[:, dc * NB:dc * NB + W], XT[dc, :, n0:n0 + W]
                    )

                hts = []
                for hc in range(4):
                    ph = pbig.tile([128, NB], F32, tag="pbig")
                    for dc in range(8):
                        nc.tensor.matmul(
                            ph[:, :W],
                            wenc[:, dc * H_ENC + hc * 128: dc * H_ENC + (hc + 1) * 128],
                            xt[:, dc * NB:dc * NB + W],
                            start=(dc == 0), stop=(dc == 7),
                        )
                    ht = hp.tile([128, NB], F32R, tag="ht")
                    nc.scalar.activation(ht[:, :W], ph[:, :W], AF.Relu,
                                         bias=benc[:, hc:hc + 1])
                    hts.append(ht)

                pz = pmisc.tile([NZ, NB], F32, tag="pmisc")
                for hc in range(4):
                    nc.tensor.matmul(
                        pz[:, :W], wz[:, hc * NZ:(hc + 1) * NZ], hts[hc][:, :W],
                        start=(hc == 0), stop=(hc == 3),
                    )
                zt2 = zp.tile([128, NB], F32R, tag="zt2")
                nc.scalar.activation(zt2[0:NZ, :W], pz[:, :W], AF.Identity, bias=bz[:])
                nc.scalar.activation(zt2[NZ:128, :W], pz[:, :W], AF.Identity, bias=bz[:])
                zsq = zp.tile([NZ, NB], F32R, tag="zsq")
                nc.vector.tensor_mul(zsq[:, :W], zt2[0:NZ, :W], zt2[0:NZ, :W])

                pd = pmisc.tile([KE, NB], F32, tag="pmisc")
                nc.tensor.matmul(pd[:, :W], bd1[:], onesn[:, :W], start=True, stop=False)
                nc.tensor.matmul(pd[:, :W], n2mu[:], zt2[0:NZ, :W], start=False, stop=False)
                nc.tensor.matmul(pd[:, :W], o64[:], zsq[:, :W], start=False, stop=True)
                qr = qp.tile([KE, NB], F32R, tag="qr")
                nc.vector.reciprocal(qr[:, :W], pd[:, :W])
                pqs = pmisc.tile([1, NB], F32, tag="pmisc")
                nc.tensor.matmul(pqs[:, :W], o16[:], qr[:, :W], start=True, stop=True)
                rqs = qp.tile([1, NB], F32R, tag="rqs")
                nc.vector.reciprocal(rqs[:, :W], pqs[:, :W])
                prb = pmisc.tile([32, NB], F32, tag="pmisc")
                nc.tensor.matmul(prb[:, :W], o132[:], rqs[:, :W], start=True, stop=True)
                prb_sb = qp.tile([32, NB], F32R, tag="prb_sb")
                nc.scalar.activation(prb_sb[:, :W], prb[:, :W], AF.Copy)
                return dict(zt2=zt2, qr=qr, prb_sb=prb_sb, n0=n0)

            def back(st):
                zt2, qr, prb_sb, n0 = st["zt2"], st["qr"], st["prb_sb"], st["n0"]
                pp = ppred.tile([32, NB], F32, tag="ppred")
                nc.tensor.matmul(pp[:, :W], late["b2p"][:], qr[:, :W], start=True, stop=False)

                ncomb = NPAIR * 4
                ci = 0
                for j in range(NPAIR):
                    if not has_b1:
                        pqbt = pqb.tile([128, NB], F32, tag="pqb")
                        nc.tensor.matmul(pqbt[:, :W], late["e2"][:, j * 128:(j + 1) * 128],
                                         qr[:, :W], start=True, stop=True)
                        zq = zqp.tile([128, NB], F32R, tag="zq")
                        nc.vector.tensor_mul(zq[:, :W], zt2[:, :W], pqbt[:, :W])
                    else:
                        zq = zt2
                    for hc in range(2):
                        for half in range(2):
                            k = 2 * j + half
                            idx = k * 2 + hc
                            pe_ = pbig.tile([128, NB], F32, tag="pbig")
                            nc.tensor.matmul(
                                pe_[:, :W],
                                late["w1p"][64 * half:64 * (half + 1),
                                    j * H_EXP + hc * 128: j * H_EXP + (hc + 1) * 128],
                                zq[64 * half:64 * (half + 1), :W],
                                start=True, stop=True,
                                tile_position=(64 * half, 0),
                            )
                            eh = ehp.tile([128, NB], F32R, tag="eh")
                            if not has_b1:
                                if idx % 8 < 5:
                                    nc.scalar.activation(eh[:, :W], pe_[:, :W], AF.Relu,
                                                         bias=0.0)
                                else:
                                    nc.vector.tensor_scalar_max(eh[:, :W], pe_[:, :W], 0.0)
                            else:
                                nc.scalar.activation(eh[:], pe_[:], AF.Relu,
                                                     bias=late["b1c"][:, idx:idx + 1])
                                pqk = pqb.tile([128, NB], F32, tag="pqb")
                                nc.tensor.matmul(pqk[:],
                                                 late["e2s"][:, k * 128:(k + 1) * 128],
                                                 qr[:], start=True, stop=True)
                                ehq = ehp.tile([128, NB], F32R, tag="ehq")
                                nc.vector.tensor_mul(ehq[:], eh[:], pqk[:])
                                eh = ehq
                            ci += 1
                            nc.tensor.matmul(
                                pp[:, :W],
                                late["w2c"][:, idx * 32:(idx + 1) * 32],
                                eh[:, :W],
                                start=False, stop=(ci == ncomb),
                                skip_group_check=True,
                            )

                ti = trp.tile([32, NB], F32, tag="ti")
                nc.scalar.activation(ti[:, :W], pp[:, :W], AF.Copy)
                nc.vector.tensor_mul(ti[:, :W], ti[:, :W], prb_sb[:, :W])
                tr = trp.tile([32, NB], F32, tag="tr")
                nc.vector.transpose(tr[:, :W], ti[:, :W])
                nc.sync.dma_start(
                    OUT[n0:n0 + W, :].rearrange("(b p) c -> p b c", p=32),
                    tr[:].rearrange("p (b v) -> p b v", v=32)[:, 0:W // 32, 0:C],
                )

            A = cfg["ahead"]
            XA = cfg["xahead"]
            for _rep in range(cfg["repeat"]):
                for ib in range(1, min(XA, NBLK)):
                    if ib not in xdma:
                        dma_x(ib)
                sts = [front(0)]
                if _rep == 0 and not late:
                    load_late_weights()
                emit_q(sts[0])
                for j in range(NPAIR):
                    emit_zq(sts[0], j)
                for ib in range(1, min(A, NBLK)):
                    sts.append(front(ib))
                    emit_q(sts[ib])
                    for j in range(NPAIR):
                        emit_zq(sts[ib], j)
                for ib in range(NBLK):
                    if ib + XA < NBLK and (ib + XA) not in xdma:
                        dma_x(ib + XA)
                    fut = None
                    if ib + A < NBLK:
                        sts.append(front(ib + A))
                        fut = sts[ib + A]
                    back(sts[ib], fut=fut, last=(ib == NBLK - 1))
                sts.clear()
                if cfg["repeat"] > 1:
                    dma_x(0)

    nc.compile()
    return nc


def _prep_v1(inputs):
    f = lambda a: np.ascontiguousarray(np.asarray(a, dtype=np.float32))
    X, enc_W, enc_b = f(inputs["X"]), f(inputs["enc_W"]), f(inputs["enc_b"])
    z_W, z_b, mu = f(inputs["z_W"]), f(inputs["z_b"]), f(inputs["mu"])
    W1, b1, W2, b2 = f(inputs["W1"]), f(inputs["b1"]), f(inputs["W2"]), f(inputs["b2"])

    has_b1 = bool(np.any(b1))

    XT = np.ascontiguousarray(X.T)
    com = {
        "Wenc": np.ascontiguousarray(
            enc_W.reshape(8, 128, H_ENC).transpose(1, 0, 2).reshape(128, 8 * H_ENC)),
        "Wz": np.ascontiguousarray(
            z_W.reshape(4, 128, NZ).transpose(1, 0, 2).reshape(128, 4 * NZ)),
        "NEG2MUT": np.ascontiguousarray(-2.0 * mu.T),
        "ONES64": np.ones((NZ, KE), np.float32),
        "ONES16": np.ones((KE, 1), np.float32),
        "ONES1_32": np.ones((1, 32), np.float32),
        "ONESN": np.ones((1, NB), np.float32),
        "BENC": np.ascontiguousarray(enc_b.reshape(4, 128).T),
        "BZ": z_b.reshape(NZ, 1).copy(),
        "BD1": (1.0 + (mu.astype(np.float64) ** 2).sum(axis=1)
                ).astype(np.float32).reshape(1, KE),
    }
    w1p = np.zeros((128, NPAIR * H_EXP), np.float32)
    e2 = np.zeros((KE, NPAIR * 128), np.float32)
    for j in range(NPAIR):
        w1p[0:64, j * H_EXP:(j + 1) * H_EXP] = W1[2 * j]
        w1p[64:128, j * H_EXP:(j + 1) * H_EXP] = W1[2 * j + 1]
        e2[2 * j, j * 128: j * 128 + 64] = 1.0
        e2[2 * j + 1, j * 128 + 64: j * 128 + 128] = 1.0
    com["W1p"], com["E2"] = w1p, e2

    w2c = np.zeros((128, KE * 2 * 32), np.float32)
    for k in range(KE):
        for hc in range(2):
            w2c[:, (k * 2 + hc) * 32:(k * 2 + hc) * 32 + C] = \
                W2[k][hc * 128:(hc + 1) * 128, :]
    com["W2c"] = w2c

    b2pad = np.zeros((KE, 32), np.float32)
    b2pad[:, 0:C] = b2
    com["B2PAD"] = b2pad

    if has_b1:
        b1c = np.zeros((128, KE * 2), np.float32)
        e2s = np.zeros((KE, KE * 128), np.float32)
        for k in range(KE):
            for hc in range(2):
                b1c[:, k * 2 + hc] = b1[k, hc * 128:(hc + 1) * 128]
            e2s[k, k * 128:(k + 1) * 128] = 1.0
        com["B1C"], com["E2S"] = b1c, e2s

    in_maps = []
    for c in range(NCORES):
        m = dict(com)
        shard = np.ascontiguousarray(XT[:, c * NS:(c + 1) * NS])
        m["XT"] = shard.reshape(8, 128, NS)
        in_maps.append(m)
    return in_maps, has_b1
